# revision 1
# baseline (speedup 1.0000x reference)
"""Causal self-attention (B=4, T=2048, C=1024, H=16, Dh=64) on 8 trn2 NeuronCores.

Sharding: core = 2*b + g  (b = batch 0..3, g = head-group 0..1, 8 heads each).
Each core computes its batch's QKV projection for its 8 heads, causal
attention, and a partial out-projection; host sums the two head-group
partials per batch (the "all-reduce" of the tensor-parallel split).

Device algorithm (per core), all matmuls in fp32r (tf32-like, 1 cyc/row):
  - x^T resident in SBUF; q^T,k^T computed as w^T-stationary matmuls
    giving [j, t] layout directly; V computed in natural [t, j] layout.
  - S^T[tk, tq] = k^T.T @ q^T per head (K=64 contraction, two heads packed
    into PE row-groups 0-63/64-127), causal tiles only.
  - additive -1e5 mask on diagonal-straddling tiles (DVE), exp on ACT
    (scale=1/8 folded in, no max-subtraction: |S|/8 <= ~9 for this data).
  - P@V with ones-augmented V (lhsT [tk,65]) -> y_aug^T[65, tq]; row 64
    accumulates the softmax denominator for free.
  - reciprocal + K=1 ones matmul broadcasts 1/rowsum across partitions;
    DVE multiply normalizes y^T.
  - out-projection from y^T tiles (lhsT [j, t]) into natural [t, e] layout.
"""

import sys

for _p in ("/opt/trn_rl_repo", "/opt/pypackages"):
    if _p not in sys.path:
        sys.path.append(_p)

import numpy as np
from contextlib import ExitStack

import concourse.bass as bass
import concourse.tile as tile
from concourse import bacc, mybir
from concourse.bass_utils import run_bass_kernel_spmd

B, T, C = 4, 2048, 1024
H, DH = 16, 64
HG = 8          # heads per core
JW = 512        # tq tile width
KW = 128        # tk tile width
NT = T // JW    # 4 tq tiles
NK = T // KW    # 16 tk tiles
NC_ = C // 128  # 8 c tiles
MASK_VAL = -1.0e5
F32 = mybir.dt.float32
F32R = mybir.dt.float32r
EXP = mybir.ActivationFunctionType.Exp

_cache = {}


def _build():
    nc = bacc.Bacc("TRN2", target_bir_lowering=False, debug=False, num_devices=8)
    xT = nc.dram_tensor("xT", [C, T], F32, kind="ExternalInput").ap()
    wqk = nc.dram_tensor("wqk", [C, 1024], F32, kind="ExternalInput").ap()
    wv = nc.dram_tensor("wv", [C, 512], F32, kind="ExternalInput").ap()
    wout = nc.dram_tensor("wout", [512, C], F32, kind="ExternalInput").ap()
    dmask = nc.dram_tensor("dmask", [128, 128], F32, kind="ExternalInput").ap()
    ones_row = nc.dram_tensor("ones_row", [1, 64], F32, kind="ExternalInput").ap()
    ones_col = nc.dram_tensor("ones_col", [128, 1], F32, kind="ExternalInput").ap()
    out = nc.dram_tensor("out", [T, C], F32, kind="ExternalOutput").ap()

    with tile.TileContext(nc) as tc:
        with ExitStack() as ctx:
            ctx.enter_context(nc.allow_low_precision(reason="fp32r rounding intended"))
            # ---- persistent SBUF tensors ----
            qk_pool = ctx.enter_context(tc.tile_pool(name="qkT", bufs=1))
            v_pool = ctx.enter_context(tc.tile_pool(name="v", bufs=1))
            const_pool = ctx.enter_context(tc.tile_pool(name="const", bufs=1))

            qk_sb = [qk_pool.tile([128, T], F32R, tag=f"qk{j}", name=f"qk_sb{j}") for j in range(8)]
            v_all = v_pool.tile([128, NK * HG * 65], F32R, tag="v_all", name="v_all")
            v_sb = [v_all[:, 520 * i:520 * i + 520] for i in range(NK)]
            onesr = const_pool.tile([1, 64], F32R, tag="onesr", name="onesr")
            onesc = const_pool.tile([128, 1], F32R, tag="onesc", name="onesc")
            nc.gpsimd.dma_start(onesr[:], ones_row[:])
            nc.gpsimd.dma_start(onesc[:], ones_col[:])

            # ================= phase 1: projections =================
            with ExitStack() as p1:
                xt_pool = p1.enter_context(tc.tile_pool(name="xt", bufs=1))
                wqk_pool = p1.enter_context(tc.tile_pool(name="wqk", bufs=16))
                wv_pool = p1.enter_context(tc.tile_pool(name="wv", bufs=1))
                pj_psum = p1.enter_context(
                    tc.tile_pool(name="pj_psum", bufs=4, space="PSUM"))

                xt = []
                for ct in range(NC_):
                    t_ = xt_pool.tile([128, T], F32R, tag=f"xt{ct}")
                    nc.gpsimd.dma_start(t_[:], xT[128 * ct:128 * ct + 128, :])
                    xt.append(t_)
                wv_sb = []
                for ct in range(NC_):
                    t_ = wv_pool.tile([128, 512], F32R, tag=f"wv{ct}")
                    nc.gpsimd.dma_start(t_[:], wv[128 * ct:128 * ct + 128, :])
                    wv_sb.append(t_)

                # q^T / k^T: out[j, t] = sum_c wqk[c, j] * xT[c, t]
                for jt in range(8):
                    wts = []
                    for ct in range(NC_):
                        w_ = wqk_pool.tile([128, 128], F32R)
                        nc.gpsimd.dma_start(
                            w_[:], wqk[128 * ct:128 * ct + 128,
                                       128 * jt:128 * jt + 128])
                        wts.append(w_)
                    for tt in range(NT):
                        ps = pj_psum.tile([128, JW], F32, tag="pjq")
                        for ct in range(NC_):
                            nc.tensor.matmul(
                                ps[:], wts[ct][:],
                                xt[ct][:, JW * tt:JW * tt + JW],
                                start=(ct == 0), stop=(ct == NC_ - 1))
                        nc.scalar.copy(qk_sb[jt][:, JW * tt:JW * tt + JW], ps[:])

                # V natural + ones column: out[t, j] = sum_c xT[c, t] * wv[c, j]
                for it in range(NK):
                    ps = pj_psum.tile([128, 512], F32, tag="pjv")
                    for ct in range(NC_):
                        nc.tensor.matmul(
                            ps[:], xt[ct][:, 128 * it:128 * it + 128],
                            wv_sb[ct][:],
                            start=(ct == 0), stop=(ct == NC_ - 1))
                    nc.scalar.copy(
                        v_sb[it][:].rearrange("p (h d) -> p h d", h=HG, d=65)[:, :, 0:64],
                        ps[:].rearrange("p (h d) -> p h d", h=HG, d=64))
                    for h in range(HG):
                        nc.vector.tensor_copy(
                            v_sb[it][:, 65 * h + 64:65 * h + 65], onesc[:])

            # ================= phase 2: attention =================
            y_pool = ctx.enter_context(tc.tile_pool(name="y", bufs=1))
            with ExitStack() as p2:
                mask_pool = p2.enter_context(tc.tile_pool(name="mask", bufs=1))
                p_pool = p2.enter_context(tc.tile_pool(name="p", bufs=10))
                fin_pool = p2.enter_context(tc.tile_pool(name="fin", bufs=3))
                s_psum = p2.enter_context(
                    tc.tile_pool(name="s_psum", bufs=3, space="PSUM"))
                y_psum = p2.enter_context(
                    tc.tile_pool(name="y_psum", bufs=2, space="PSUM"))
                bc_psum = p2.enter_context(
                    tc.tile_pool(name="bc_psum", bufs=1, space="PSUM"))

                dmask_sb = mask_pool.tile([128, 128], F32, tag="dm", name="dmask_sb")
                nc.sync.dma_start(dmask_sb[:], dmask[:])
                y_sb = [y_pool.tile([128, T], F32R, tag=f"y{m}", name=f"y_sb{m}") for m in range(4)]

                for m in range(4):          # head pairs (2m, 2m+1)
                    for J in range(NT):     # tq tiles
                        psy = {0: y_psum.tile([65, JW], F32, tag="ya", name="psya"),
                               64: y_psum.tile([65, JW], F32, tag="yb", name="psyb")}
                        nki = 4 * J + 4     # causal tk tiles
                        # reversed: diagonal (straddling, narrowed) tiles first;
                        # start=True on the first clears the whole psy bank, so
                        # later full-width matmuls overwrite-where-unwritten.
                        order = list(reversed(range(nki)))
                        CH = 4
                        for c0 in range(0, nki, CH):
                            chunk = order[c0:c0 + CH]
                            Ps = {}
                            # S run: uniform K=64 row-group pairs, back-to-back
                            Ss = {}
                            for i in chunk:
                                r = i - 4 * J
                                lo = 128 * r if r > 0 else 0
                                for off in (0, 64):
                                    S = s_psum.tile([128, JW], F32, tag="s", name="S")
                                    nc.tensor.matmul(
                                        S[:, lo:JW],
                                        qk_sb[4 + m][off:off + 64, 128 * i:128 * i + 128],
                                        qk_sb[m][off:off + 64, JW * J + lo:JW * J + JW],
                                        start=True, stop=True)
                                    Ss[(i, off)] = (S, lo)
                                if r >= 0:
                                    for off in (0, 64):
                                        nc.vector.tensor_add(
                                            Ss[(i, off)][0][:, 128 * r:128 * r + 128],
                                            Ss[(i, off)][0][:, 128 * r:128 * r + 128],
                                            dmask_sb[:])
                                for off in (0, 64):
                                    S, lo_ = Ss[(i, off)]
                                    P = p_pool.tile([128, JW], F32R, tag="p", name="P")
                                    nc.scalar.activation(
                                        P[:, lo_:JW], S[:, lo_:JW], EXP, scale=0.125)
                                    Ps[(i, off)] = (P, lo_)
                            # PV run: uniform K=128 matmuls, back-to-back
                            for i in chunk:
                                for off in (0, 64):
                                    h = 2 * m + (1 if off else 0)
                                    P, lo_ = Ps[(i, off)]
                                    nc.tensor.matmul(
                                        psy[off][:, lo_:JW],
                                        v_sb[i][:, 65 * h:65 * h + 65],
                                        P[:, lo_:JW],
                                        start=(i == order[0]),
                                        stop=(i == order[-1]))
                        for off in (0, 64):
                            # rowsum -> f32r (ACT), broadcast via K=1 matmul,
                            # approx-reciprocal, multiply into y^T
                            rsr = fin_pool.tile([1, JW], F32R, tag="rsr", name="rsr")
                            nc.vector.tensor_copy(rsr[:], psy[off][64:65, :])
                            bc = bc_psum.tile([64, JW], F32, tag="bc", name="bc")
                            nc.tensor.matmul(bc[:], onesr[:], rsr[:],
                                             start=True, stop=True)
                            rec = fin_pool.tile([64, JW], F32, tag="rec", name="rec")
                            nc.vector.reciprocal_approx_fast(rec[:], bc[:])
                            nc.vector.tensor_mul(
                                y_sb[m][off:off + 64, JW * J:JW * J + JW],
                                psy[off][0:64, :], rec[:])

            # ================= phase 3: out projection =================
            with ExitStack() as p3:
                wo_pool = p3.enter_context(tc.tile_pool(name="wo", bufs=1))
                o_pool = p3.enter_context(tc.tile_pool(name="o", bufs=4))
                o_psum = p3.enter_context(
                    tc.tile_pool(name="o_psum", bufs=4, space="PSUM"))

                wo_sb = {}
                for jt in range(4):
                    for et in range(2):
                        w_ = wo_pool.tile([128, 512], F32R, tag=f"wo{jt}{et}")
                        nc.gpsimd.dma_start(
                            w_[:], wout[128 * jt:128 * jt + 128,
                                        512 * et:512 * et + 512])
                        wo_sb[(jt, et)] = w_
                for it in range(NK):
                    for et in range(2):
                        ps = o_psum.tile([128, 512], F32, tag="ops")
                        for jt in range(4):
                            nc.tensor.matmul(
                                ps[:],
                                y_sb[jt][:, 128 * it:128 * it + 128],
                                wo_sb[(jt, et)][:],
                                start=(jt == 0), stop=(jt == 3))
                        ot = o_pool.tile([128, 512], F32, tag="ot")
                        nc.scalar.copy(ot[:], ps[:])
                        nc.sync.dma_start(
                            out[128 * it:128 * it + 128,
                                512 * et:512 * et + 512], ot[:])
    nc.compile()
    return nc


def _host_masks():
    a = np.arange(128, dtype=np.int64)[:, None]
    b = np.arange(128, dtype=np.int64)[None, :]
    return np.where(a <= b, np.float32(0.0), np.float32(MASK_VAL))


def _make_in_map(core, x, w_qkv, w_out):
    b, g = divmod(core, 2)
    xT = np.ascontiguousarray(x[b].T)
    wqk = np.ascontiguousarray(np.concatenate(
        [w_qkv[:, 512 * g:512 * g + 512],
         w_qkv[:, 1024 + 512 * g:1024 + 512 * g + 512]], axis=1))
    wv = np.ascontiguousarray(w_qkv[:, 2048 + 512 * g:2048 + 512 * g + 512])
    wout_s = np.ascontiguousarray(w_out[512 * g:512 * g + 512, :])
    return dict(xT=xT, wqk=wqk, wv=wv, wout=wout_s,
                dmask=_host_masks(),
                ones_row=np.ones((1, 64), np.float32),
                ones_col=np.ones((128, 1), np.float32))


def kernel(x, w_qkv, w_out):
    x = np.ascontiguousarray(x, dtype=np.float32)
    w_qkv = np.ascontiguousarray(w_qkv, dtype=np.float32)
    w_out = np.ascontiguousarray(w_out, dtype=np.float32)

    if "nc" not in _cache:
        _cache["nc"] = _build()
    nc = _cache["nc"]

    in_maps = [_make_in_map(core, x, w_qkv, w_out) for core in range(8)]

    res = run_bass_kernel_spmd(nc, in_maps, core_ids=list(range(8)))
    out = np.empty((B, T, C), np.float32)
    for b in range(B):
        out[b] = res.results[2 * b]["out"] + res.results[2 * b + 1]["out"]
    return out



# revision 3
# speedup vs baseline: 1.7163x; 1.7163x over previous
"""Causal self-attention (B=4, T=2048, C=1024, H=16, Dh=64) on 8 trn2 NeuronCores.

Sharding: core = 2*b + g  (b = batch 0..3, g = head-group 0..1, 8 heads each).
Each core computes its batch's QKV projection for its 8 heads, causal
attention, and a partial out-projection; host sums the two head-group
partials per batch (the "all-reduce" of the tensor-parallel split).

Device algorithm (per core), all matmuls in bf16 (2 cols/cycle streaming):
  - x^T resident in SBUF (bf16, host-cast); q^T,k^T as w-stationary matmuls
    giving [j, t] layout; V natural [t, j] with a ones column per head
    (rowsum accumulates in the PV matmul for free).
  - S^T[tk, tq] = k^T.T @ q^T per head (K=64), causal tiles only, written
    into 2-bank PSUM slabs ([128,1024] = both heads of a pair for one tk
    tile); ONE trimmed exp per slab (ACT per-instruction overhead ~310cyc
    amortized), scale=1/8 folded in, no max-subtraction.
  - diagonal-straddle masking via 0/1 triangular-mask multiply on the P
    (bf16, SBUF) tile on the otherwise-idle GpSimd engine.
  - PV with ones-augmented V (lhsT [tk,65]) -> y_aug^T[65, tq].
  - reciprocal + K=1 ones matmul broadcasts 1/rowsum across partitions;
    DVE multiply normalizes y^T (bf16).
  - out-projection from y^T tiles into natural [t, e] layout, bf16 out,
    host up-casts and sums the pair partials.
  - qk-projection / out-projection chunks are interleaved into the
    attention stream as PE filler so the PE never idles long enough for
    the HAM clock gate to re-throttle (the old kernel ran its whole
    attention phase at 1.2 GHz because of this).
"""

import sys

for _p in ("/opt/trn_rl_repo", "/opt/pypackages"):
    if _p not in sys.path:
        sys.path.append(_p)

import numpy as np
from contextlib import ExitStack

import concourse.bass as bass
import concourse.tile as tile
from concourse import bacc, mybir
from concourse.bass_utils import run_bass_kernel_spmd

import ml_dtypes

BF16NP = np.dtype(ml_dtypes.bfloat16)

B, T, C = 4, 2048, 1024
H, DH = 16, 64
HG = 8          # heads per core
JW = 512        # tq tile width
NT = T // JW    # 4 tq tiles
NK = T // 128   # 16 tk tiles
F32 = mybir.dt.float32
BF = mybir.dt.bfloat16
EXP = mybir.ActivationFunctionType.Exp

_cache = {}


def _build():
    nc = bacc.Bacc("TRN2", target_bir_lowering=False, debug=False, num_devices=8)
    xT = nc.dram_tensor("xT", [C, T], BF, kind="ExternalInput").ap()
    wqk = nc.dram_tensor("wqk", [C, 1024], BF, kind="ExternalInput").ap()
    wv = nc.dram_tensor("wv", [C, 512], BF, kind="ExternalInput").ap()
    wout = nc.dram_tensor("wout", [512, C], BF, kind="ExternalInput").ap()
    trimask = nc.dram_tensor("trimask", [128, 128], BF, kind="ExternalInput").ap()
    ones_row = nc.dram_tensor("ones_row", [1, 64], BF, kind="ExternalInput").ap()
    out = nc.dram_tensor("out", [T, C], BF, kind="ExternalOutput").ap()

    with tile.TileContext(nc) as tc:
        with ExitStack() as ctx:
            ctx.enter_context(nc.allow_low_precision(reason="bf16 matmuls intended"))
            sb = ctx.enter_context(tc.tile_pool(name="sb", bufs=1))
            ppool = ctx.enter_context(tc.tile_pool(name="ppool", bufs=4))
            small = ctx.enter_context(tc.tile_pool(name="small", bufs=2))
            otp = ctx.enter_context(tc.tile_pool(name="otp", bufs=2))
            # PSUM: slab 2x2 banks + psy 2 + bc 1 + pj 1 = 8 banks exactly
            slab_p = ctx.enter_context(tc.tile_pool(name="slab_p", bufs=2, space="PSUM"))
            psy_p = ctx.enter_context(tc.tile_pool(name="psy_p", bufs=2, space="PSUM"))
            bc_p = ctx.enter_context(tc.tile_pool(name="bc_p", bufs=1, space="PSUM"))
            pj_p = ctx.enter_context(tc.tile_pool(name="pj_p", bufs=1, space="PSUM"))

            # ---- persistent SBUF ----
            xt_all = sb.tile([128, 8 * T], BF, tag="xt")
            wv_all = sb.tile([128, 8 * 512], BF, tag="wv")
            wqk_all = sb.tile([128, 8 * 1024], BF, tag="wqk")
            wout_all = sb.tile([128, 4 * 1024], BF, tag="wout")
            qk_sb = [sb.tile([128, T], BF, tag=f"qk{j}", name=f"qk{j}") for j in range(8)]
            v_all = sb.tile([128, NK * HG * 65], BF, tag="v")
            y_sb = [sb.tile([128, T], BF, tag=f"y{m}", name=f"y{m}") for m in range(4)]
            tri_sb = sb.tile([128, 128], BF, tag="tri")
            onesr_sb = sb.tile([1, 64], BF, tag="onesr")
            scratch = sb.tile([1, 64], BF, tag="scratch")

            xt = lambda ct: xt_all[:, T * ct:T * (ct + 1)]
            wvt = lambda ct: wv_all[:, 512 * ct:512 * (ct + 1)]
            wqkt = lambda ct, jt: wqk_all[:, 1024 * ct + 128 * jt:1024 * ct + 128 * jt + 128]
            woutt = lambda jt, et: wout_all[:, 1024 * jt + 512 * et:1024 * jt + 512 * et + 512]
            vt = lambda i: v_all[:, 520 * i:520 * (i + 1)]

            # ---- input DMAs (few big transfers, spread across queues) ----
            nc.sync.dma_start(tri_sb[:], trimask[:])
            nc.sync.dma_start(onesr_sb[:], ones_row[:])
            # preload the exp table set while DMAs run
            nc.scalar.activation(scratch[:], onesr_sb[:], EXP, scale=0.125)
            nc.gpsimd.memset(v_all[:], 1.0)  # ones columns; V chunks overwrite
            nc.sync.dma_start(
                xt_all[:].rearrange("p (c t) -> p c t", c=8),
                xT[:].rearrange("(c p) t -> p c t", p=128))
            nc.gpsimd.dma_start(
                wv_all[:].rearrange("p (c j) -> p c j", c=8),
                wv[:].rearrange("(c p) j -> p c j", p=128))
            nc.gpsimd.dma_start(
                wqk_all[:].rearrange("p (c j) -> p c j", c=8),
                wqk[:].rearrange("(c p) j -> p c j", p=128))
            nc.gpsimd.dma_start(
                wout_all[:].rearrange("p (j e) -> p j e", j=4),
                wout[:].rearrange("(j p) e -> p j e", p=128))

            # ---- V projection (natural [t, j] + ones cols preserved) ----
            for it in range(NK):
                ps = slab_p.tile([128, 512], F32, tag="slab", name="psv")
                for ct in range(8):
                    nc.tensor.matmul(ps[:], xt(ct)[:, 128 * it:128 * it + 128],
                                     wvt(ct), start=(ct == 0), stop=(ct == 7))
                nc.vector.tensor_copy(
                    vt(it).rearrange("p (h d) -> p h d", h=HG, d=65)[:, :, 0:64],
                    ps[:].rearrange("p (h d) -> p h d", h=HG, d=64))

            # ---- qk projection helper ----
            def proj_chunk(jt, tt, pool, on_act):
                ps = pool.tile([128, 512], F32, tag="slab" if pool is slab_p else "pj",
                               name="psqk")
                for ct in range(8):
                    nc.tensor.matmul(ps[:], wqkt(ct, jt),
                                     xt(ct)[:, JW * tt:JW * tt + JW],
                                     start=(ct == 0), stop=(ct == 7))
                dst = qk_sb[jt][:, JW * tt:JW * tt + JW]
                if on_act:
                    nc.scalar.copy(dst, ps[:])
                else:
                    nc.vector.tensor_copy(dst, ps[:])

            # pair 0 upfront (ACT idle here)
            for jt in (0, 4):
                for tt in range(NT):
                    proj_chunk(jt, tt, slab_p, on_act=True)

            # filler: qk proj chunks for pairs 1..3, fed into attention(m-1)
            fill_proj = [(m + 1 + 4 * half, tt)
                         for m in range(3) for tt in range(NT) for half in (0, 1)]
            fill_by_m = {m: fill_proj[8 * m:8 * m + 8] for m in range(3)}

            # out-projection chunk (it, et): y^T . wout -> out[t, e]
            def out_chunk(it, et, ot, pool, on_act):
                ps = pool.tile([128, 512], F32, tag="slab" if pool is slab_p else "pj",
                               name="psout")
                for jt in range(4):
                    nc.tensor.matmul(ps[:], y_sb[jt][:, 128 * it:128 * it + 128],
                                     woutt(jt, et), start=(jt == 0), stop=(jt == 3))
                dst = ot[:, 512 * et:512 * et + 512]
                if on_act:
                    nc.scalar.copy(dst, ps[:])
                else:
                    nc.vector.tensor_copy(dst, ps[:])

            ot_tiles = {}

            # ---- attention: m-outer, J-inner ----
            for m in range(4):
                filler = list(fill_by_m.get(m, []))
                for J in range(NT):
                    nki = 4 * J + 4
                    psy = {off: psy_p.tile([65, JW], F32, tag="psy",
                                           name=f"psy{off}")
                           for off in (0, 1)}
                    prev = None
                    for i in range(nki):
                        r = i - 4 * J
                        lo = 128 * r if r > 0 else 0
                        slab = slab_p.tile([128, 1024], F32, tag="slab", name="slab")
                        for off in (0, 1):
                            nc.tensor.matmul(
                                slab[:, 512 * off + lo:512 * off + 512],
                                qk_sb[4 + m][64 * off:64 * off + 64,
                                             128 * i:128 * i + 128],
                                qk_sb[m][64 * off:64 * off + 64,
                                         JW * J + lo:JW * J + JW],
                                start=True, stop=True)
                        P = ppool.tile([128, 1024], BF, tag="p", name="P")
                        if lo:
                            nc.scalar.activation(
                                P[:].rearrange("p (o c) -> p o c", o=2)[:, :, lo:],
                                slab[:].rearrange("p (o c) -> p o c", o=2)[:, :, lo:],
                                EXP, scale=0.125)
                        else:
                            nc.scalar.activation(P[:], slab[:], EXP, scale=0.125)
                        if r >= 0:
                            for off in (0, 1):
                                blk = P[:, 512 * off + lo:512 * off + lo + 128]
                                nc.gpsimd.tensor_mul(blk, blk, tri_sb[:])
                        # PV of the previous slab (its exp has had a slab's
                        # worth of PE time to finish)
                        if prev is not None:
                            pi, plo, pP = prev
                            for off in (0, 1):
                                nc.tensor.matmul(
                                    psy[off][:, plo:JW],
                                    vt(pi)[:, 65 * (2 * m + off):65 * (2 * m + off) + 65],
                                    pP[:, 512 * off + plo:512 * off + 512],
                                    start=(pi == 0), stop=(pi == nki - 1))
                        if i % 2 == 1 and filler:
                            jt, tt = filler.pop(0)
                            proj_chunk(jt, tt, pj_p, on_act=False)
                        prev = (i, lo, P)
                    pi, plo, pP = prev
                    for off in (0, 1):
                        nc.tensor.matmul(
                            psy[off][:, plo:JW],
                            vt(pi)[:, 65 * (2 * m + off):65 * (2 * m + off) + 65],
                            pP[:, 512 * off + plo:512 * off + 512],
                            start=(pi == 0), stop=(pi == nki - 1))
                    # normalize: rowsum row 64 -> reciprocal broadcast -> y^T
                    for off in (0, 1):
                        rsr = small.tile([1, JW], BF, tag="rsr", name="rsr")
                        nc.vector.tensor_copy(rsr[:], psy[off][64:65, :])
                        bc = bc_p.tile([64, JW], F32, tag="bc", name="bc")
                        nc.tensor.matmul(bc[:], onesr_sb[:], rsr[:],
                                         start=True, stop=True)
                        rec = small.tile([64, JW], F32, tag="rec", name="rec")
                        nc.vector.reciprocal_approx_fast(rec[:], bc[:])
                        nc.vector.tensor_mul(
                            y_sb[m][64 * off:64 * off + 64, JW * J:JW * J + JW],
                            psy[off][0:64, :], rec[:])
                    # m=3: out-projection for the finished J block as filler
                    if m == 3:
                        for it in range(4 * J, 4 * J + 4):
                            ot = otp.tile([128, 1024], BF, tag="ot", name="ot")
                            ot_tiles[it] = ot
                            for et in range(2):
                                out_chunk(it, et, ot, pj_p, on_act=False)
                            nc.sync.dma_start(out[128 * it:128 * it + 128, :], ot[:])
    nc.compile()
    return nc


def _host_trimask():
    p = np.arange(128, dtype=np.int64)[:, None]
    c = np.arange(128, dtype=np.int64)[None, :]
    return (c >= p).astype(np.float32).astype(BF16NP)


def _make_in_map(core, x, w_qkv, w_out):
    b, g = divmod(core, 2)
    xT = np.ascontiguousarray(x[b].T).astype(BF16NP)
    wqk = np.ascontiguousarray(np.concatenate(
        [w_qkv[:, 512 * g:512 * g + 512],
         w_qkv[:, 1024 + 512 * g:1024 + 512 * g + 512]], axis=1)).astype(BF16NP)
    wv = np.ascontiguousarray(
        w_qkv[:, 2048 + 512 * g:2048 + 512 * g + 512]).astype(BF16NP)
    wout_s = np.ascontiguousarray(w_out[512 * g:512 * g + 512, :]).astype(BF16NP)
    return dict(xT=xT, wqk=wqk, wv=wv, wout=wout_s,
                trimask=_host_trimask(),
                ones_row=np.ones((1, 64), np.float32).astype(BF16NP))


def kernel(x, w_qkv, w_out):
    x = np.ascontiguousarray(x, dtype=np.float32)
    w_qkv = np.ascontiguousarray(w_qkv, dtype=np.float32)
    w_out = np.ascontiguousarray(w_out, dtype=np.float32)

    if "nc" not in _cache:
        _cache["nc"] = _build()
    nc = _cache["nc"]

    in_maps = [_make_in_map(core, x, w_qkv, w_out) for core in range(8)]

    res = run_bass_kernel_spmd(nc, in_maps, core_ids=list(range(8)))
    out = np.empty((B, T, C), np.float32)
    for b in range(B):
        out[b] = (np.asarray(res.results[2 * b]["out"]).astype(np.float32)
                  + np.asarray(res.results[2 * b + 1]["out"]).astype(np.float32))
    return out


# revision 11
# speedup vs baseline: 1.7346x; 1.0107x over previous
"""Causal self-attention (B=4, T=2048, C=1024, H=16, Dh=64) on 8 trn2 NeuronCores.

Sharding: core = 2*b + g  (b = batch 0..3, g = head-group 0..1, 8 heads each).
Each core computes its batch's QKV projection for its 8 heads, causal
attention, and a partial out-projection; host sums the two head-group
partials per batch (the "all-reduce" of the tensor-parallel split).

Device algorithm (per core), all matmuls in bf16 (2 cols/cycle streaming):
  - x^T resident in SBUF (bf16, host-cast); q^T,k^T as w-stationary matmuls
    giving [j, t] layout; V natural [t, j] with a ones column per head
    (rowsum accumulates in the PV matmul for free).
  - S^T[tk, tq] = k^T.T @ q^T per head (K=64), causal tiles only, written
    into 2-bank PSUM slabs ([128,1024] = both heads of a pair for one tk
    tile); ONE trimmed exp per slab (ACT per-instruction overhead ~310cyc
    amortized), scale=1/8 folded in, no max-subtraction.
  - diagonal-straddle masking via 0/1 triangular-mask multiply on the P
    (bf16, SBUF) tile on the otherwise-idle GpSimd engine.
  - PV with ones-augmented V (lhsT [tk,65]) -> y_aug^T[65, tq].
  - reciprocal + K=1 ones matmul broadcasts 1/rowsum across partitions;
    DVE multiply normalizes y^T (bf16).
  - out-projection from y^T tiles into natural [t, e] layout, bf16 out,
    host up-casts and sums the pair partials.
  - qk-projection / out-projection chunks are interleaved into the
    attention stream as PE filler so the PE never idles long enough for
    the HAM clock gate to re-throttle (the old kernel ran its whole
    attention phase at 1.2 GHz because of this).
"""

import sys

for _p in ("/opt/trn_rl_repo", "/opt/pypackages"):
    if _p not in sys.path:
        sys.path.append(_p)

import numpy as np
from contextlib import ExitStack

import concourse.bass as bass
import concourse.tile as tile
from concourse import bacc, mybir
from concourse.bass_utils import run_bass_kernel_spmd

import ml_dtypes

BF16NP = np.dtype(ml_dtypes.bfloat16)

B, T, C = 4, 2048, 1024
H, DH = 16, 64
HG = 8          # heads per core
JW = 512        # tq tile width
NT = T // JW    # 4 tq tiles
NK = T // 128   # 16 tk tiles
F32 = mybir.dt.float32
BF = mybir.dt.bfloat16
EXP = mybir.ActivationFunctionType.Exp

_cache = {}


def _build():
    nc = bacc.Bacc("TRN2", target_bir_lowering=False, debug=False, num_devices=8)
    xT = nc.dram_tensor("xT", [C, T], BF, kind="ExternalInput").ap()
    wqk = nc.dram_tensor("wqk", [C, 1024], BF, kind="ExternalInput").ap()
    wv = nc.dram_tensor("wv", [C, 512], BF, kind="ExternalInput").ap()
    wout = nc.dram_tensor("wout", [512, C], BF, kind="ExternalInput").ap()
    trimask = nc.dram_tensor("trimask", [128, 128], BF, kind="ExternalInput").ap()
    ones_row = nc.dram_tensor("ones_row", [1, 64], BF, kind="ExternalInput").ap()
    out = nc.dram_tensor("out", [T, C], BF, kind="ExternalOutput").ap()

    with tile.TileContext(nc) as tc:
        with ExitStack() as ctx:
            ctx.enter_context(nc.allow_low_precision(reason="bf16 matmuls intended"))
            sb = ctx.enter_context(tc.tile_pool(name="sb", bufs=1))
            ppool = ctx.enter_context(tc.tile_pool(name="ppool", bufs=4))
            small = ctx.enter_context(tc.tile_pool(name="small", bufs=2))
            otp = ctx.enter_context(tc.tile_pool(name="otp", bufs=2))
            # PSUM: slab 2x2 banks + psy 2 + bc 1 + pj 1 = 8 banks exactly
            slab_p = ctx.enter_context(tc.tile_pool(name="slab_p", bufs=2, space="PSUM"))
            psy_p = ctx.enter_context(tc.tile_pool(name="psy_p", bufs=2, space="PSUM"))
            bc_p = ctx.enter_context(tc.tile_pool(name="bc_p", bufs=1, space="PSUM"))
            pj_p = ctx.enter_context(tc.tile_pool(name="pj_p", bufs=1, space="PSUM"))

            # ---- persistent SBUF ----
            xt_all = sb.tile([128, 8 * T], BF, tag="xt")
            wv_all = sb.tile([128, 8 * 512], BF, tag="wv")
            wqk_all = sb.tile([128, 8 * 1024], BF, tag="wqk")
            wout_all = sb.tile([128, 4 * 1024], BF, tag="wout")
            qk_sb = [sb.tile([128, T], BF, tag=f"qk{j}", name=f"qk{j}") for j in range(8)]
            v_all = sb.tile([128, NK * HG * 65], BF, tag="v")
            y_sb = [sb.tile([128, T], BF, tag=f"y{m}", name=f"y{m}") for m in range(4)]
            tri_sb = sb.tile([128, 128], BF, tag="tri")
            onesr_sb = sb.tile([1, 64], BF, tag="onesr")
            scratch = sb.tile([1, 64], BF, tag="scratch")

            xt = lambda ct: xt_all[:, T * ct:T * (ct + 1)]
            wvt = lambda ct: wv_all[:, 512 * ct:512 * (ct + 1)]
            wqkt = lambda ct, jt: wqk_all[:, 1024 * ct + 128 * jt:1024 * ct + 128 * jt + 128]
            woutt = lambda jt, et: wout_all[:, 1024 * jt + 512 * et:1024 * jt + 512 * et + 512]
            vt = lambda i: v_all[:, 520 * i:520 * (i + 1)]

            # ---- input DMAs, ordered by need (V proj wants wv + xt ct0 first;
            # per-ct xt DMAs let the first accumulation chain chase the
            # transfers instead of waiting for the full 4MB) ----
            nc.gpsimd.dma_start(
                wv_all[:].rearrange("p (c j) -> p c j", c=8),
                wv[:].rearrange("(c p) j -> p c j", p=128))
            for ct in range(8):
                nc.sync.dma_start(xt_all[:, T * ct:T * (ct + 1)],
                                  xT[128 * ct:128 * ct + 128, :])
            nc.gpsimd.dma_start(
                wqk_all[:].rearrange("p (c j) -> p c j", c=8),
                wqk[:].rearrange("(c p) j -> p c j", p=128))
            nc.gpsimd.dma_start(
                wout_all[:].rearrange("p (j e) -> p j e", j=4),
                wout[:].rearrange("(j p) e -> p j e", p=128))
            nc.gpsimd.dma_start(tri_sb[:], trimask[:])
            nc.gpsimd.dma_start(onesr_sb[:], ones_row[:])
            # preload the exp table set while DMAs run
            nc.scalar.activation(scratch[:], onesr_sb[:], EXP, scale=0.125)
            nc.vector.memset(v_all[:], 1.0)  # ones columns; V chunks overwrite

            # ---- V projection (natural [t, j] + ones cols preserved) ----
            for it in range(NK):
                ps = slab_p.tile([128, 512], F32, tag="slab", name="psv")
                for ct in range(8):
                    nc.tensor.matmul(ps[:], xt(ct)[:, 128 * it:128 * it + 128],
                                     wvt(ct), start=(ct == 0), stop=(ct == 7))
                nc.vector.tensor_copy(
                    vt(it).rearrange("p (h d) -> p h d", h=HG, d=65)[:, :, 0:64],
                    ps[:].rearrange("p (h d) -> p h d", h=HG, d=64))

            # ---- qk projection helper ----
            def proj_chunk(jt, tt, pool, on_act):
                ps = pool.tile([128, 512], F32, tag="slab" if pool is slab_p else "pj",
                               name="psqk")
                for ct in range(8):
                    nc.tensor.matmul(ps[:], wqkt(ct, jt),
                                     xt(ct)[:, JW * tt:JW * tt + JW],
                                     start=(ct == 0), stop=(ct == 7))
                dst = qk_sb[jt][:, JW * tt:JW * tt + JW]
                if on_act:
                    nc.scalar.copy(dst, ps[:])
                else:
                    nc.vector.tensor_copy(dst, ps[:])

            # pair 0 upfront (ACT idle here)
            for jt in (0, 4):
                for tt in range(NT):
                    proj_chunk(jt, tt, slab_p, on_act=True)

            # filler: qk proj chunks for pairs 1..3, fed into attention(m-1)
            fill_proj = [(m + 1 + 4 * half, tt)
                         for m in range(3) for tt in range(NT) for half in (0, 1)]
            fill_by_m = {m: fill_proj[8 * m:8 * m + 8] for m in range(3)}

            # out-projection chunk (it, et): y^T . wout -> out[t, e]
            ot_tiles = {}

            def out_chunk(it, et, pool, on_act):
                if it not in ot_tiles:
                    ot_tiles[it] = otp.tile([128, 1024], BF, tag="ot", name="ot")
                ot = ot_tiles[it]
                tag = {id(slab_p): "slab", id(pj_p): "pj", id(bc_p): "bc"}[id(pool)]
                ps = pool.tile([128, 512], F32, tag=tag, name="psout")
                for jt in range(4):
                    nc.tensor.matmul(ps[:], y_sb[jt][:, 128 * it:128 * it + 128],
                                     woutt(jt, et), start=(jt == 0), stop=(jt == 3))
                dst = ot[:, 512 * et:512 * et + 512]
                if on_act:
                    nc.scalar.copy(dst, ps[:])
                else:
                    nc.vector.tensor_copy(dst, ps[:])
                if et == 1:
                    nc.sync.dma_start(out[128 * it:128 * it + 128, :], ot[:])

            out_queue = []

            # ---- attention: m-outer, J-inner ----
            n_out = [0]
            for m in range(4):
                filler = list(fill_by_m.get(m, []))
                for J in range(NT):
                    nki = 4 * J + 4
                    psy = {off: psy_p.tile([65, JW], F32, tag="psy",
                                           name=f"psy{off}")
                           for off in (0, 1)}
                    prev = None
                    for i in range(nki):
                        r = i - 4 * J
                        lo = 128 * r if r > 0 else 0
                        slab = slab_p.tile([128, 1024], F32, tag="slab", name="slab")
                        for off in (0, 1):
                            nc.tensor.matmul(
                                slab[:, 512 * off + lo:512 * off + 512],
                                qk_sb[4 + m][64 * off:64 * off + 64,
                                             128 * i:128 * i + 128],
                                qk_sb[m][64 * off:64 * off + 64,
                                         JW * J + lo:JW * J + JW],
                                start=True, stop=True)
                        P = ppool.tile([128, 1024], BF, tag="p", name="P")
                        if lo:
                            nc.scalar.activation(
                                P[:].rearrange("p (o c) -> p o c", o=2)[:, :, lo:],
                                slab[:].rearrange("p (o c) -> p o c", o=2)[:, :, lo:],
                                EXP, scale=0.125)
                        else:
                            nc.scalar.activation(P[:], slab[:], EXP, scale=0.125)
                        if r >= 0:
                            for off in (0, 1):
                                blk = P[:, 512 * off + lo:512 * off + lo + 128]
                                nc.gpsimd.tensor_mul(blk, blk, tri_sb[:])
                        # PV of the previous slab (its exp has had a slab's
                        # worth of PE time to finish)
                        if prev is not None:
                            pi, plo, pP = prev
                            for off in (0, 1):
                                nc.tensor.matmul(
                                    psy[off][:, plo:JW],
                                    vt(pi)[:, 65 * (2 * m + off):65 * (2 * m + off) + 65],
                                    pP[:, 512 * off + plo:512 * off + 512],
                                    start=(pi == 0), stop=(pi == nki - 1))
                        if i % 2 == 1 and filler:
                            jt, tt = filler.pop(0)
                            proj_chunk(jt, tt, pj_p, on_act=False)
                        elif out_queue:
                            oit, oet = out_queue.pop(0)
                            n_out[0] += 1
                            out_chunk(oit, oet, bc_p if n_out[0] % 2 else pj_p,
                                      on_act=False)
                        prev = (i, lo, P)
                    pi, plo, pP = prev
                    for off in (0, 1):
                        nc.tensor.matmul(
                            psy[off][:, plo:JW],
                            vt(pi)[:, 65 * (2 * m + off):65 * (2 * m + off) + 65],
                            pP[:, 512 * off + plo:512 * off + 512],
                            start=(pi == 0), stop=(pi == nki - 1))
                    # normalize: rowsum row 64 -> reciprocal broadcast -> y^T
                    # the two bc matmuls go to col-groups 0-1 / 2-3 and run
                    # concurrently on the PE
                    rsrs = {}
                    for off in (0, 1):
                        rsr = small.tile([1, JW], BF, tag="rsr", name="rsr")
                        nc.vector.tensor_copy(rsr[:], psy[off][64:65, :])
                        rsrs[off] = rsr
                    bc = bc_p.tile([128, JW], F32, tag="bc", name="bc")
                    nc.tensor.matmul(bc[0:64, :], onesr_sb[:], rsrs[0][:],
                                     start=True, stop=True)
                    nc.tensor.matmul(bc[64:128, :], onesr_sb[:], rsrs[1][:],
                                     start=True, stop=True, tile_position=(0, 64))
                    rec = small.tile([128, JW], F32, tag="rec", name="rec")
                    nc.vector.reciprocal_approx_fast(rec[:], bc[:])
                    for off in (0, 1):
                        nc.vector.tensor_mul(
                            y_sb[m][64 * off:64 * off + 64, JW * J:JW * J + JW],
                            psy[off][0:64, :], rec[64 * off:64 * off + 64, :])
                    # m=3: queue out-projection for the finished J block; it
                    # drains one chunk per slab through the filler slots
                    if m == 3:
                        for it in range(4 * J, 4 * J + 4):
                            out_queue.extend([(it, 0), (it, 1)])
            # drain remaining out-projection chunks, alternating psum pools so
            # the copies overlap the matmul chains
            for n, (oit, oet) in enumerate(out_queue):
                out_chunk(oit, oet, slab_p if n % 2 else pj_p, on_act=(n % 2 == 0))
    nc.compile()
    return nc


def _host_trimask():
    p = np.arange(128, dtype=np.int64)[:, None]
    c = np.arange(128, dtype=np.int64)[None, :]
    return (c >= p).astype(np.float32).astype(BF16NP)


def _make_in_map(core, x, w_qkv, w_out):
    b, g = divmod(core, 2)
    xT = np.ascontiguousarray(x[b].T).astype(BF16NP)
    wqk = np.ascontiguousarray(np.concatenate(
        [w_qkv[:, 512 * g:512 * g + 512],
         w_qkv[:, 1024 + 512 * g:1024 + 512 * g + 512]], axis=1)).astype(BF16NP)
    wv = np.ascontiguousarray(
        w_qkv[:, 2048 + 512 * g:2048 + 512 * g + 512]).astype(BF16NP)
    wout_s = np.ascontiguousarray(w_out[512 * g:512 * g + 512, :]).astype(BF16NP)
    return dict(xT=xT, wqk=wqk, wv=wv, wout=wout_s,
                trimask=_host_trimask(),
                ones_row=np.ones((1, 64), np.float32).astype(BF16NP))


def kernel(x, w_qkv, w_out):
    x = np.ascontiguousarray(x, dtype=np.float32)
    w_qkv = np.ascontiguousarray(w_qkv, dtype=np.float32)
    w_out = np.ascontiguousarray(w_out, dtype=np.float32)

    if "nc" not in _cache:
        _cache["nc"] = _build()
    nc = _cache["nc"]

    in_maps = [_make_in_map(core, x, w_qkv, w_out) for core in range(8)]

    res = run_bass_kernel_spmd(nc, in_maps, core_ids=list(range(8)))
    out = np.empty((B, T, C), np.float32)
    for b in range(B):
        out[b] = (np.asarray(res.results[2 * b]["out"]).astype(np.float32)
                  + np.asarray(res.results[2 * b + 1]["out"]).astype(np.float32))
    return out


# revision 15
# speedup vs baseline: 1.8359x; 1.0584x over previous
"""Causal self-attention (B=4, T=2048, C=1024, H=16, Dh=64) on 8 trn2 NeuronCores.

Sharding: core = 2*b + g  (b = batch 0..3, g = head-group 0..1, 8 heads each).
Each core computes its batch's QKV projection for its 8 heads, causal
attention, and a partial out-projection; host sums the two head-group
partials per batch (the "all-reduce" of the tensor-parallel split).

Device algorithm (per core), all matmuls in bf16 (2 cols/cycle streaming):
  - x^T resident in SBUF (bf16, host-cast); q^T,k^T as w-stationary matmuls
    giving [j, t] layout; V natural [t, j] with a ones column per head
    (rowsum accumulates in the PV matmul for free).
  - S^T[tk, tq] = k^T.T @ q^T per head (K=64), causal tiles only, written
    into 2-bank PSUM slabs ([128,1024] = both heads of a pair for one tk
    tile); ONE trimmed exp per slab (ACT per-instruction overhead ~310cyc
    amortized), scale=1/8 folded in, no max-subtraction.
  - diagonal-straddle masking via 0/1 triangular-mask multiply on the P
    (bf16, SBUF) tile on the otherwise-idle GpSimd engine.
  - PV with ones-augmented V (lhsT [tk,65]) -> y_aug^T[65, tq].
  - reciprocal + K=1 ones matmul broadcasts 1/rowsum across partitions;
    DVE multiply normalizes y^T (bf16).
  - out-projection from y^T tiles into natural [t, e] layout, bf16 out,
    host up-casts and sums the pair partials.
  - qk-projection / out-projection chunks are interleaved into the
    attention stream as PE filler so the PE never idles long enough for
    the HAM clock gate to re-throttle (the old kernel ran its whole
    attention phase at 1.2 GHz because of this).
"""

import sys

for _p in ("/opt/trn_rl_repo", "/opt/pypackages"):
    if _p not in sys.path:
        sys.path.append(_p)

import numpy as np
from contextlib import ExitStack

import concourse.bass as bass
import concourse.tile as tile
from concourse import bacc, mybir
from concourse.bass_utils import run_bass_kernel_spmd

import ml_dtypes

BF16NP = np.dtype(ml_dtypes.bfloat16)

B, T, C = 4, 2048, 1024
H, DH = 16, 64
HG = 8          # heads per core
JW = 512        # tq tile width
NT = T // JW    # 4 tq tiles
NK = T // 128   # 16 tk tiles
F32 = mybir.dt.float32
BF = mybir.dt.bfloat16
EXP = mybir.ActivationFunctionType.Exp

_cache = {}


def _build():
    nc = bacc.Bacc("TRN2", target_bir_lowering=False, debug=False, num_devices=8)
    xT = nc.dram_tensor("xT", [C, T], BF, kind="ExternalInput").ap()
    wqk = nc.dram_tensor("wqk", [C, 1024], BF, kind="ExternalInput").ap()
    wv = nc.dram_tensor("wv", [C, 512], BF, kind="ExternalInput").ap()
    wout = nc.dram_tensor("wout", [512, C], BF, kind="ExternalInput").ap()
    trimask = nc.dram_tensor("trimask", [128, 128], BF, kind="ExternalInput").ap()
    ones_row = nc.dram_tensor("ones_row", [1, 64], BF, kind="ExternalInput").ap()
    out = nc.dram_tensor("out", [T, C], BF, kind="ExternalOutput").ap()

    with tile.TileContext(nc) as tc:
        with ExitStack() as ctx:
            ctx.enter_context(nc.allow_low_precision(reason="bf16 matmuls intended"))
            sb = ctx.enter_context(tc.tile_pool(name="sb", bufs=1))
            ppool = ctx.enter_context(tc.tile_pool(name="ppool", bufs=4))
            small = ctx.enter_context(tc.tile_pool(name="small", bufs=2))
            otp = ctx.enter_context(tc.tile_pool(name="otp", bufs=2))
            # PSUM: slab 2x2 banks + psy 2 + bc 1 + pj 1 = 8 banks exactly
            slab_p = ctx.enter_context(tc.tile_pool(name="slab_p", bufs=2, space="PSUM"))
            psy_p = ctx.enter_context(tc.tile_pool(name="psy_p", bufs=2, space="PSUM"))
            bc_p = ctx.enter_context(tc.tile_pool(name="bc_p", bufs=1, space="PSUM"))
            pj_p = ctx.enter_context(tc.tile_pool(name="pj_p", bufs=1, space="PSUM"))

            # ---- persistent SBUF ----
            xt_all = sb.tile([128, 8 * T], BF, tag="xt")
            wv_all = sb.tile([128, 8 * 512], BF, tag="wv")
            wqk_all = sb.tile([128, 8 * 1024], BF, tag="wqk")
            wout_all = sb.tile([128, 4 * 1024], BF, tag="wout")
            qk_sb = [sb.tile([128, T], BF, tag=f"qk{j}", name=f"qk{j}") for j in range(8)]
            v_all = sb.tile([128, NK * HG * 65], BF, tag="v")
            y_sb = [sb.tile([128, T], BF, tag=f"y{m}", name=f"y{m}") for m in range(4)]
            tri_sb = sb.tile([128, 128], BF, tag="tri")
            onesr_sb = sb.tile([1, 64], BF, tag="onesr")
            scratch = sb.tile([1, 64], BF, tag="scratch")

            xt = lambda ct: xt_all[:, T * ct:T * (ct + 1)]
            wvt = lambda ct: wv_all[:, 512 * ct:512 * (ct + 1)]
            wqkt = lambda ct, jt: wqk_all[:, 1024 * ct + 128 * jt:1024 * ct + 128 * jt + 128]
            woutt = lambda jt, et: wout_all[:, 1024 * jt + 512 * et:1024 * jt + 512 * et + 512]
            vt = lambda i: v_all[:, 520 * i:520 * (i + 1)]

            # ---- input DMAs, ordered by need (V proj wants wv + xt ct0 first;
            # per-ct xt DMAs let the first accumulation chain chase the
            # transfers instead of waiting for the full 4MB) ----
            nc.gpsimd.dma_start(
                wv_all[:].rearrange("p (c j) -> p c j", c=8),
                wv[:].rearrange("(c p) j -> p c j", p=128))
            # xt arrives in T-chunks: each chunk carries ALL c-tiles for a
            # 256-col t-range, so the V projection (which contracts over all
            # of c per t-tile) can start after the first chunk lands
            for tch in range(8):
                nc.sync.dma_start(
                    xt_all[:].rearrange("p (c t) -> p c t", c=8)
                    [:, :, 256 * tch:256 * tch + 256],
                    xT[:].rearrange("(c p) t -> p c t", p=128)
                    [:, :, 256 * tch:256 * tch + 256])
            nc.gpsimd.dma_start(
                wqk_all[:].rearrange("p (c j) -> p c j", c=8),
                wqk[:].rearrange("(c p) j -> p c j", p=128))
            nc.gpsimd.dma_start(
                wout_all[:].rearrange("p (j e) -> p j e", j=4),
                wout[:].rearrange("(j p) e -> p j e", p=128))
            nc.gpsimd.dma_start(tri_sb[:], trimask[:])
            nc.gpsimd.dma_start(onesr_sb[:], ones_row[:])
            # preload the exp table set while DMAs run
            nc.scalar.activation(scratch[:], onesr_sb[:], EXP, scale=0.125)
            nc.vector.memset(v_all[:], 1.0)  # ones columns; V chunks overwrite

            # ---- V projection (natural [t, j] + ones cols preserved) ----
            for it in range(NK):
                ps = slab_p.tile([128, 512], F32, tag="slab", name="psv")
                for ct in range(8):
                    nc.tensor.matmul(ps[:], xt(ct)[:, 128 * it:128 * it + 128],
                                     wvt(ct), start=(ct == 0), stop=(ct == 7))
                nc.vector.tensor_copy(
                    vt(it).rearrange("p (h d) -> p h d", h=HG, d=65)[:, :, 0:64],
                    ps[:].rearrange("p (h d) -> p h d", h=HG, d=64))

            # ---- qk projection helper ----
            def proj_chunk(jt, tt, pool, on_act):
                ps = pool.tile([128, 512], F32, tag="slab" if pool is slab_p else "pj",
                               name="psqk")
                for ct in range(8):
                    nc.tensor.matmul(ps[:], wqkt(ct, jt),
                                     xt(ct)[:, JW * tt:JW * tt + JW],
                                     start=(ct == 0), stop=(ct == 7))
                dst = qk_sb[jt][:, JW * tt:JW * tt + JW]
                if on_act:
                    nc.scalar.copy(dst, ps[:])
                else:
                    nc.vector.tensor_copy(dst, ps[:])

            # only the pair-0 tt=0 blocks upfront; everything else streams in
            # as filler between attention slabs
            proj_chunk(0, 0, slab_p, on_act=True)
            proj_chunk(4, 0, slab_p, on_act=True)

            # staggered filler schedule: (m, J) emits the chunks attention
            # needs 1+ J-blocks later, so every block (incl. (m, J0)) has PE
            # filler while ACT grinds through the exps
            def fillers_for(m, J):
                out = []
                if J == 0:
                    out += [(m, 3), (m + 4, 3)]          # own pair's tt=3
                if m < 3 and J >= 1:
                    out += [(m + 1, J - 1), (m + 5, J - 1)]
                if m == 0 and J <= 1:
                    out += [(0, J + 1), (4, J + 1)]
                return out

            # out-projection chunk (it, et): y^T . wout -> out[t, e]
            ot_tiles = {}

            def out_chunk(it, et, pool, on_act):
                if it not in ot_tiles:
                    ot_tiles[it] = otp.tile([128, 1024], BF, tag="ot", name="ot")
                ot = ot_tiles[it]
                tag = {id(slab_p): "slab", id(pj_p): "pj", id(bc_p): "bc"}[id(pool)]
                ps = pool.tile([128, 512], F32, tag=tag, name="psout")
                for jt in range(4):
                    nc.tensor.matmul(ps[:], y_sb[jt][:, 128 * it:128 * it + 128],
                                     woutt(jt, et), start=(jt == 0), stop=(jt == 3))
                dst = ot[:, 512 * et:512 * et + 512]
                if on_act:
                    nc.scalar.copy(dst, ps[:])
                else:
                    nc.vector.tensor_copy(dst, ps[:])
                if et == 1:
                    nc.sync.dma_start(out[128 * it:128 * it + 128, :], ot[:])

            out_queue = []

            # ---- attention: m-outer, J-inner ----
            n_out = [0]
            for m in range(4):
                filler = []
                for J in range(NT):
                    filler.extend(fillers_for(m, J))
                    nki = 4 * J + 4
                    psy = {off: psy_p.tile([65, JW], F32, tag="psy",
                                           name=f"psy{off}")
                           for off in (0, 1)}
                    prev = None
                    for i in range(nki):
                        r = i - 4 * J
                        lo = 128 * r if r > 0 else 0
                        slab = slab_p.tile([128, 1024], F32, tag="slab", name="slab")
                        for off in (0, 1):
                            nc.tensor.matmul(
                                slab[:, 512 * off + lo:512 * off + 512],
                                qk_sb[4 + m][64 * off:64 * off + 64,
                                             128 * i:128 * i + 128],
                                qk_sb[m][64 * off:64 * off + 64,
                                         JW * J + lo:JW * J + JW],
                                start=True, stop=True)
                        P = ppool.tile([128, 1024], BF, tag="p", name="P")
                        if lo:
                            nc.scalar.activation(
                                P[:].rearrange("p (o c) -> p o c", o=2)[:, :, lo:],
                                slab[:].rearrange("p (o c) -> p o c", o=2)[:, :, lo:],
                                EXP, scale=0.125)
                        else:
                            nc.scalar.activation(P[:], slab[:], EXP, scale=0.125)
                        if r >= 0:
                            for off in (0, 1):
                                blk = P[:, 512 * off + lo:512 * off + lo + 128]
                                nc.gpsimd.tensor_mul(blk, blk, tri_sb[:])
                        # PV of the previous slab (its exp has had a slab's
                        # worth of PE time to finish)
                        if prev is not None:
                            pi, plo, pP = prev
                            for off in (0, 1):
                                nc.tensor.matmul(
                                    psy[off][:, plo:JW],
                                    vt(pi)[:, 65 * (2 * m + off):65 * (2 * m + off) + 65],
                                    pP[:, 512 * off + plo:512 * off + 512],
                                    start=(pi == 0), stop=(pi == nki - 1))
                        if filler and (i % 2 == 1
                                       or len(filler) > (nki - i) // 2):
                            jt, tt = filler.pop(0)
                            proj_chunk(jt, tt, pj_p, on_act=False)
                        elif out_queue:
                            oit, oet = out_queue.pop(0)
                            n_out[0] += 1
                            out_chunk(oit, oet, bc_p if n_out[0] % 2 else pj_p,
                                      on_act=False)
                        prev = (i, lo, P)
                    pi, plo, pP = prev
                    for off in (0, 1):
                        nc.tensor.matmul(
                            psy[off][:, plo:JW],
                            vt(pi)[:, 65 * (2 * m + off):65 * (2 * m + off) + 65],
                            pP[:, 512 * off + plo:512 * off + 512],
                            start=(pi == 0), stop=(pi == nki - 1))
                    # normalize: rowsum row 64 -> reciprocal broadcast -> y^T
                    # the two bc matmuls go to col-groups 0-1 / 2-3 and run
                    # concurrently on the PE
                    rsrs = {}
                    for off in (0, 1):
                        rsr = small.tile([1, JW], BF, tag="rsr", name="rsr")
                        nc.vector.tensor_copy(rsr[:], psy[off][64:65, :])
                        rsrs[off] = rsr
                    bc = bc_p.tile([128, JW], F32, tag="bc", name="bc")
                    nc.tensor.matmul(bc[0:64, :], onesr_sb[:], rsrs[0][:],
                                     start=True, stop=True)
                    nc.tensor.matmul(bc[64:128, :], onesr_sb[:], rsrs[1][:],
                                     start=True, stop=True, tile_position=(0, 64))
                    rec = small.tile([128, JW], F32, tag="rec", name="rec")
                    nc.vector.reciprocal_approx_fast(rec[:], bc[:])
                    for off in (0, 1):
                        nc.vector.tensor_mul(
                            y_sb[m][64 * off:64 * off + 64, JW * J:JW * J + JW],
                            psy[off][0:64, :], rec[64 * off:64 * off + 64, :])
                    # m=3: queue out-projection for the finished J block; it
                    # drains one chunk per slab through the filler slots
                    if m == 3:
                        for it in range(4 * J, 4 * J + 4):
                            out_queue.extend([(it, 0), (it, 1)])
            # drain remaining out-projection chunks, alternating psum pools so
            # the copies overlap the matmul chains
            for n, (oit, oet) in enumerate(out_queue):
                out_chunk(oit, oet, slab_p if n % 2 else pj_p, on_act=(n % 2 == 0))
    nc.compile()
    return nc


def _host_trimask():
    p = np.arange(128, dtype=np.int64)[:, None]
    c = np.arange(128, dtype=np.int64)[None, :]
    return (c >= p).astype(np.float32).astype(BF16NP)


def _make_in_map(core, x, w_qkv, w_out):
    b, g = divmod(core, 2)
    xT = np.ascontiguousarray(x[b].T).astype(BF16NP)
    wqk = np.ascontiguousarray(np.concatenate(
        [w_qkv[:, 512 * g:512 * g + 512],
         w_qkv[:, 1024 + 512 * g:1024 + 512 * g + 512]], axis=1)).astype(BF16NP)
    wv = np.ascontiguousarray(
        w_qkv[:, 2048 + 512 * g:2048 + 512 * g + 512]).astype(BF16NP)
    wout_s = np.ascontiguousarray(w_out[512 * g:512 * g + 512, :]).astype(BF16NP)
    return dict(xT=xT, wqk=wqk, wv=wv, wout=wout_s,
                trimask=_host_trimask(),
                ones_row=np.ones((1, 64), np.float32).astype(BF16NP))


def kernel(x, w_qkv, w_out):
    x = np.ascontiguousarray(x, dtype=np.float32)
    w_qkv = np.ascontiguousarray(w_qkv, dtype=np.float32)
    w_out = np.ascontiguousarray(w_out, dtype=np.float32)

    if "nc" not in _cache:
        _cache["nc"] = _build()
    nc = _cache["nc"]

    in_maps = [_make_in_map(core, x, w_qkv, w_out) for core in range(8)]

    res = run_bass_kernel_spmd(nc, in_maps, core_ids=list(range(8)))
    out = np.empty((B, T, C), np.float32)
    for b in range(B):
        out[b] = (np.asarray(res.results[2 * b]["out"]).astype(np.float32)
                  + np.asarray(res.results[2 * b + 1]["out"]).astype(np.float32))
    return out
